# revision 9
# baseline (speedup 1.0000x reference)
# CGAT (graph attention) Trainium2 Bass kernel.
#
# Reference computation (B=2, V=8192, T=8, F0=F1=32):
#   h  = lrelu(einsum('bvtf,gf->bvtg', x, W_w) + W_b)
#   ht = mean_b(einsum('bvtg,t->bvg', h, Wt_w))
#   s_src = ht @ a_w[:32] ; s_dst = ht @ a_w[32:]
#   e = lrelu(s_dst[:,None] + s_src[None,:]) ; masked by adj>0 ; softmax over j
#   out = lrelu(attn @ h)
#
# Distribution: destination rows i sharded across 8 cores (1024 each); each
# core redundantly computes h for ALL v (cheap), then its (1024, 8192)
# masked-softmax aggregation.  Inputs are rolled per core by the shard
# offset so the SPMD program is core-independent; no collectives.
#
# Score algebra: softmax over j is invariant to any per-row-i factor, so
# with z = s_dst[i]+s_src[j] we aggregate with
#   p[j,i] = exp(lrelu(z) - s_dst[i]) * adj = max(e1[j], G[i]*rc2[j]) * adj
# where e1=exp(s_src), rc2=exp(0.2 s_src), G=exp(-0.8 s_dst).
# Z[i] = sum_j p[j,i] via a ones-weights PE matmul (one per j-block, no
# per-slice weight reloads); out = Prelu(out_ps * (1/Z)) in one ACT op.
#
# Inner loop engine split per (i-half, j-block):
#   DVE:  p = max(e1[j], G_rep*rc2[j]) * adjT   (ONE fused custom-DVE op)
#   PE :  out_psum += p.T @ h (4 mm) ; z_row += ones.T @ p (1 mm)
# Phase 1: h evacuated with a single ACT Prelu per v-block (alpha honored
# by HW); s_src/s_dst via one fused multiply+accum reduction per v-block,
# alternating DVE STT-accum / (DVE mult + ACT Copy-accum) to balance
# engines.  Phase 3 pass 0 is interleaved with phase 1 per v-chunk group
# so the PE stays warm and DVE/ACT/PE overlap throughout.

import os

import numpy as np
import ml_dtypes

B, V, T, F0, F1 = 2, 8192, 8, 32, 32
NCORES = 8
SHARD = V // NCORES          # 1024 destination rows per core
NVB = V // 128               # 64 v-blocks (j blocks)
BTG = B * T * F1             # 512 feature columns of h
NPASS = 2                    # i handled in 2 halves of 512 rows (PSUM budget)
IHALF = SHARD // NPASS       # 512
JGRP = 8                     # adj DMA groups 8 j-blocks (1 MB per transfer)
ALPHA = 0.2

_prog_cache = {}
_pmask_op = None


def _get_pmask_op():
    # Fused DVE op: out = max(s0, in0*s1) * in1  (score clamp + adjacency
    # mask in one 1x pass; replaces an ACT relu + a DVE STT).
    global _pmask_op
    if _pmask_op is not None:
        return _pmask_op
    from concourse import dve_ops
    from concourse.dve_spec import Spec, Src0, Src1, C0, C1, maxx, lower
    from concourse.dve_uop import DveOpSpec

    name = "P_MASK_ANT"
    if name not in dve_ops._SUB_OPCODE_FOR_NAME:
        spec = Spec(
            body=maxx(C0, Src0 * C1) * Src1,
            reference=lambda in0, in1, s0, s1: np.maximum(s0, in0 * s1) * in1,
        )
        opcode = max(dve_ops._SUB_OPCODE_FOR_NAME.values()) + 1
        assert opcode < 0x20
        shas = {}
        for ver in ("v3", "v4"):
            try:
                shas[ver] = DveOpSpec(
                    name=name, opcode=opcode, uops=lower(spec, ver=ver),
                    rd1_en=True).sha(ver)
            except Exception:
                pass
        op = dve_ops.DveOp(name, spec, subdim=False, uops_sha=shas)
        dve_ops.OPS.append(op)
        dve_ops.CUSTOM_DVE_SPECS[name] = spec
        dve_ops._SUB_OPCODE_FOR_NAME[name] = opcode
        _pmask_op = op
    else:
        _pmask_op = next(o for o in dve_ops.OPS if o.name == name)
    return _pmask_op


def _build_program(with_bias: bool):
    import concourse.bacc as bacc
    import concourse.mybir as mybir
    import concourse.tile as tile

    pmask = _get_pmask_op()

    nc = bacc.Bacc("TRN2", target_bir_lowering=False, debug=False,
                   num_devices=NCORES)
    f32 = mybir.dt.float32
    bf16 = mybir.dt.bfloat16
    AF = mybir.ActivationFunctionType
    OP = mybir.AluOpType

    # host-packed layouts (all contiguous per DMA tile):
    #   xt[b, tq, vcg, (t4 f)=128, 2048]   (4 v-chunks of 512 per DMA)
    #   adjt[pass, jbg, jb8, j=128, i=512] (adj_s transposed, 8 jb per DMA)
    x_d = nc.dram_tensor("xt", [B, 2, 4, 128, 2048], bf16,
                         kind="ExternalInput").ap()
    adj_d = nc.dram_tensor("adjt", [NPASS, NVB // JGRP, JGRP, 128, IHALF],
                           bf16, kind="ExternalInput").ap()
    wblk_d = nc.dram_tensor("wblk", [128, 128], bf16, kind="ExternalInput").ap()
    wa1_d = nc.dram_tensor("wa1", [128, BTG], bf16, kind="ExternalInput").ap()
    wa2_d = nc.dram_tensor("wa2", [128, BTG], bf16, kind="ExternalInput").ap()
    out_d = nc.dram_tensor("out", [B, SHARD, T, F1], f32,
                           kind="ExternalOutput").ap()
    if with_bias:
        wb_d = nc.dram_tensor("wb", [1, BTG], f32, kind="ExternalInput").ap()
    sd_scr = nc.dram_tensor("sd_scr", [SHARD], f32, kind="Internal").ap()
    z_scr = nc.dram_tensor("z_scr", [NPASS, IHALF], f32, kind="Internal").ap()

    with tile.TileContext(nc) as tc:
        with (
            tc.tile_pool(name="consts", bufs=1) as consts,
            tc.tile_pool(name="xb", bufs=9) as xb_pool,
            tc.tile_pool(name="hps", bufs=2, space="PSUM") as hps_pool,
            tc.tile_pool(name="ssc", bufs=4) as ssc_pool,
            tc.tile_pool(name="adjp", bufs=7) as adj_pool,
            tc.tile_pool(name="ptp", bufs=10) as pt_pool,
            tc.tile_pool(name="ops", bufs=5, space="PSUM") as out_ps_pool,
            tc.tile_pool(name="zps", bufs=1, space="PSUM") as z_ps_pool,
            tc.tile_pool(name="fin", bufs=3) as fin_pool,
        ):
            # ---- constants ----
            wblk = consts.tile([128, 128], bf16)
            nc.sync.dma_start(wblk[:], wblk_d)
            wa1 = consts.tile([128, BTG], bf16)
            nc.sync.dma_start(wa1[:], wa1_d)
            wa2 = consts.tile([128, BTG], bf16)
            nc.sync.dma_start(wa2[:], wa2_d)
            ones = consts.tile([128, 1], bf16)
            nc.gpsimd.memset(ones[:], 1.0)
            if with_bias:
                wb_sb = consts.tile([128, BTG], f32)
                nc.gpsimd.dma_start(wb_sb[:], wb_d.partition_broadcast(128))

            # persistent tensors
            h_sb = consts.tile([128, NVB * BTG], bf16)   # h, [v, (b t4 g)]
            ssrc = consts.tile([128, NVB], f32)          # s_src per node
            sdst = consts.tile([128, SHARD // 128], f32)  # s_dst own shard
            grep = consts.tile([128, SHARD], bf16)       # exp(-.8 s_dst) rep
            e1c = consts.tile([128, NVB], f32)           # exp(s_src)
            rc2 = consts.tile([128, NVB], f32)           # exp(0.2 s_src)
            rz = consts.tile([128, NPASS * 4], f32)      # 1/Z per i block

            # ---- phase 1 for one v-chunk group (16 v-blocks) ----
            def phase1(vcg):
                xbs = []
                for b in range(B):
                    for tq in range(2):
                        xb = xb_pool.tile([128, 2048], bf16,
                                          tag="xb", name=f"xb_{vcg}_{b}_{tq}")
                        nc.sync.dma_start(xb[:], x_d[b, tq, vcg])
                        xbs.append(xb)
                for vc4 in range(4):
                    for vb4 in range(4):
                        vblk = vcg * 16 + vc4 * 4 + vb4
                        hps = hps_pool.tile([128, BTG], f32)
                        for bt in range(4):
                            c0 = vc4 * 512 + vb4 * 128
                            nc.tensor.matmul(
                                hps[:, bt * 128:(bt + 1) * 128],
                                lhsT=xbs[bt][:, c0:c0 + 128],
                                rhs=wblk[:],
                                start=True, stop=True)
                        if with_bias:
                            nc.vector.scalar_tensor_tensor(
                                hps[:], hps[:], 1.0, wb_sb[:],
                                op0=OP.bypass, op1=OP.add)
                        hsl = h_sb[:, vblk * BTG:(vblk + 1) * BTG]
                        # single-op lrelu evacuation (Prelu honors alpha)
                        nc.scalar.activation(hsl, hps[:], AF.Prelu,
                                             bias=0.0, scale=1.0, alpha=ALPHA)
                        # fused s = sum_c h*wa; alternate DVE-only / DVE+ACT
                        def s_op(dst_col, wa, k):
                            sc = ssc_pool.tile([128, BTG], bf16, tag="sc",
                                               name=f"sc{k}_{vblk}")
                            if (vblk + k) % 8 >= 1:
                                nc.vector.scalar_tensor_tensor(
                                    sc[:], hsl, 1.0, wa[:],
                                    op0=OP.mult, op1=OP.mult,
                                    accum_out=dst_col)
                            else:
                                nc.vector.tensor_tensor(sc[:], hsl, wa[:],
                                                        op=OP.mult)
                                nc.scalar.activation(
                                    sc[:], sc[:], AF.Copy, bias=0.0,
                                    scale=1.0, accum_out=dst_col)
                        s_op(ssrc[:, vblk:vblk + 1], wa1, 0)
                        if vblk < SHARD // 128:
                            s_op(sdst[:, vblk:vblk + 1], wa2, 1)

            def exps(vcg):
                lo, hi = vcg * 16, (vcg + 1) * 16
                nc.scalar.activation(e1c[:, lo:hi], ssrc[:, lo:hi], AF.Exp,
                                     bias=0.0, scale=1.0)
                nc.scalar.activation(rc2[:, lo:hi], ssrc[:, lo:hi], AF.Exp,
                                     bias=0.0, scale=0.2)

            def bcast_dma():
                # s_dst (own shard) across partitions via a DRAM roundtrip
                nc.gpsimd.dma_start(
                    sd_scr.rearrange("(c p) -> p c", p=128), sdst[:])
                sdrep = consts.tile([128, SHARD], f32)
                nc.gpsimd.dma_start(sdrep[:],
                                    sd_scr.partition_broadcast(128))
                return sdrep

            # ---- phase 3: one pass accumulates out/Z over a jb range ----
            state = {}

            def p3_open(ip):
                out_ps = [out_ps_pool.tile([128, BTG], f32, tag="out_ps",
                                           name=f"out_ps_{ip}_{k}")
                          for k in range(4)]
                z_row = z_ps_pool.tile([1, IHALF], f32, tag="z_row",
                                       name=f"z_row_{ip}")
                state[ip] = (out_ps, z_row)

            def p3_run(ip, jbg_lo, jbg_hi):
                i0 = ip * IHALF
                gh = grep[:, i0:i0 + IHALF]
                out_ps, z_row = state[ip]
                for jbg in range(jbg_lo, jbg_hi):
                    adjq = adj_pool.tile([128, JGRP, IHALF], bf16, tag="adjq",
                                         name=f"adjq_{ip}_{jbg}")
                    dma_eng = nc.scalar if jbg % 2 == 0 else nc.gpsimd
                    dma_eng.dma_start(
                        adjq[:], adj_d[ip, jbg].rearrange("k p f -> p k f"))
                    for jb8 in range(JGRP):
                        jb = jbg * JGRP + jb8
                        # p = max(e1[j], G*rc2[j]) * adjT  (one fused DVE op)
                        pt = pt_pool.tile([128, IHALF], bf16, tag="pt",
                                          name=f"pt_{ip}_{jb}")
                        nc.vector._custom_dve(
                            pmask, out=pt[:], in0=gh, in1=adjq[:, jb8, :],
                            s0=e1c[:, jb:jb + 1], s1=rc2[:, jb:jb + 1])
                        first, last = jb == 0, jb == NVB - 1
                        for ib in range(4):
                            nc.tensor.matmul(
                                out_ps[ib][:],
                                lhsT=pt[:, ib * 128:(ib + 1) * 128],
                                rhs=h_sb[:, jb * BTG:(jb + 1) * BTG],
                                start=first, stop=last)
                        # Z row: ones.T @ p  (one mm, trivial weight load)
                        nc.tensor.matmul(z_row[:], lhsT=ones[:], rhs=pt[:],
                                         start=first, stop=last)

            def p3_close(ip):
                i0 = ip * IHALF
                out_ps, z_row = state[ip]
                # transpose Z row to [i-part, 1] via DRAM roundtrip
                zsb = fin_pool.tile([1, IHALF], f32, tag="zsb",
                                    name=f"zsb_{ip}")
                nc.scalar.copy(zsb[:], z_row[:])
                nc.gpsimd.dma_start(z_scr[ip], zsb[0, :])
                zt = fin_pool.tile([128, 4], f32, tag="zt", name=f"zt_{ip}")
                nc.gpsimd.dma_start(
                    zt[:], z_scr[ip].rearrange("(c p) -> p c", p=128))
                rzs_all = rz[:, ip * 4:ip * 4 + 4]
                nc.vector.reciprocal(rzs_all, zt[:])
                for ib in range(4):
                    rzs = rz[:, ip * 4 + ib:ip * 4 + ib + 1]
                    fin = fin_pool.tile([128, BTG], f32, tag="fin",
                                        name=f"fin_{ip}_{ib}")
                    # out = prelu(out_psum * (1/Z)) in one ACT op
                    nc.scalar.activation(fin[:], out_ps[ib][:], AF.Prelu,
                                         bias=0.0, scale=rzs, alpha=ALPHA)
                    r0 = i0 + ib * 128
                    dst = out_d[:, r0:r0 + 128, :, :].rearrange(
                        "b i t g -> i b t g")
                    nc.gpsimd.dma_start(
                        dst, fin[:].rearrange("i (b t g) -> i b t g", b=B, t=T))

            # ---- schedule: interleave pass 0 with phase 1 per vcg ----
            phase1(0)
            sdrep = bcast_dma()          # DMAs start early (gpsimd queue)
            phase1(1)                    # scalar queue stays on evacuations
            # grep ACT lands after phase1(1) evacs; roundtrip long done
            nc.scalar.activation(grep[:], sdrep[:], AF.Exp, bias=0.0,
                                 scale=-0.8)
            exps(0)
            exps(1)
            p3_open(0)
            p3_run(0, 0, 2)      # jb 0..15 (vcg 0)
            phase1(2)
            p3_run(0, 2, 4)      # jb 16..31
            exps(2)
            phase1(3)
            p3_run(0, 4, 6)      # jb 32..47
            exps(3)
            p3_run(0, 6, 8)      # jb 48..63
            p3_close(0)
            p3_open(1)
            p3_run(1, 0, 8)
            p3_close(1)

    nc.compile()
    return nc


def _host_constants(W_w, W_b, Wt_w, a_w):
    bf = ml_dtypes.bfloat16
    # W_blk[(t4,f),(t4,g)] = W_w[g,f] block-diag x4
    wblk = np.zeros((128, 128), np.float32)
    wT = np.asarray(W_w, np.float32).T           # [f, g]
    for t in range(4):
        wblk[t * 32:(t + 1) * 32, t * 32:(t + 1) * 32] = wT
    # wa[c] for c=(b,t,g): (Wt_w[t]/B) * a_w[g]
    wt = np.asarray(Wt_w, np.float64) / B
    a1 = np.asarray(a_w[:F1], np.float64)
    a2 = np.asarray(a_w[F1:], np.float64)
    c_t = np.tile(np.repeat(wt, F1), B)
    wa1 = (c_t * np.tile(a1, B * T)).astype(np.float32)
    wa2 = (c_t * np.tile(a2, B * T)).astype(np.float32)
    wa1_rep = np.broadcast_to(wa1.astype(bf), (128, BTG)).copy()
    wa2_rep = np.broadcast_to(wa2.astype(bf), (128, BTG)).copy()
    return wblk.astype(bf), wa1_rep, wa2_rep


def _make_in_maps(x, W_w, W_b, Wt_w, a_w, adj, with_bias):
    wblk, wa1_rep, wa2_rep = _host_constants(W_w, W_b, Wt_w, a_w)
    bf = ml_dtypes.bfloat16
    # x to [b, tq, (t4 f)=128, v] then packed [b, tq, vcg, 128, 2048]
    xt_base = np.ascontiguousarray(
        x.reshape(B, V, 2, 128).transpose(0, 2, 3, 1)).astype(bf)
    adj_t = np.ascontiguousarray(adj.T)          # [j, i_global] int32

    in_maps = []
    for c in range(NCORES):
        s = c * SHARD
        xt_roll = np.roll(xt_base, -s, axis=3)
        xt = np.ascontiguousarray(
            xt_roll.reshape(B, 2, 128, 4, 2048).transpose(0, 1, 3, 2, 4))
        at = adj_t[:, s:s + SHARD]               # [j, i] for shard rows
        adj_ts = np.concatenate([at[s:, :], at[:s, :]], axis=0)  # roll j
        # [pass, jbg, jb8, 128, IHALF]
        adjt = np.ascontiguousarray(
            adj_ts.reshape(NVB // JGRP, JGRP, 128, NPASS, IHALF)
            .transpose(3, 0, 1, 2, 4)).astype(bf)
        m = {
            "xt": xt,
            "adjt": adjt,
            "wblk": wblk,
            "wa1": wa1_rep,
            "wa2": wa2_rep,
        }
        if with_bias:
            wb_rep = np.tile(np.asarray(W_b, np.float32), B * T)[None, :]
            m["wb"] = np.ascontiguousarray(wb_rep)
        in_maps.append(m)
    return in_maps


def kernel(x, W_w, W_b, Wt_w, a_w, adj):
    from concourse.bass_utils import run_bass_kernel_spmd

    x = np.ascontiguousarray(np.asarray(x, np.float32))
    adj = np.ascontiguousarray(np.asarray(adj, np.int32))
    with_bias = bool(np.any(np.asarray(W_b) != 0))
    if with_bias not in _prog_cache:
        _prog_cache[with_bias] = _build_program(with_bias)
    nc = _prog_cache[with_bias]

    in_maps = _make_in_maps(x, W_w, W_b, Wt_w, a_w, adj, with_bias)

    trace = os.environ.get("KERNEL_TRACE", "0") == "1"
    res = run_bass_kernel_spmd(nc, in_maps, core_ids=list(range(NCORES)),
                               trace=trace)
    kernel.last_results = res
    out = np.concatenate([r["out"] for r in res.results], axis=1)
    return out.astype(np.float32)


kernel.last_results = None
